# revision 66
# baseline (speedup 1.0000x reference)
"""MultiHeadAttention Trainium2 kernel (8 NeuronCores, data-parallel over batch).

Contract: kernel(**inputs) takes the FULL inputs from setup_inputs() and
returns the FULL [8, 512, 1024] output. Batch element c runs on NeuronCore c
(B == n_cores == 8); no collectives.

Per-core computation (batch b, S=512, D=1024, H=16, Dk=64), all matmul
operands fp16 (same 10-bit mantissa as fp32r, half the HBM traffic):
  QT = (w_q/8)^T-proj of query^T  -> [D, S]  (head h rows h*64..h*64+63)
  KT likewise (unscaled)          -> [D, S]
  V  = natural value proj         -> [S, D]  (stored with a ones column per head)
  per head: scoresT[k,q'] = KT_h-block^T @ QT_h   (q' = reversed query index)
            attnT = exp(scoresT) * emt[h]         (emt = host exp(rel-bias) *
                    {0,1} mask -- fp16*fp16 runs DVE at 2x, masking is exact)
            ctxT[65,S] = [V_h | 1]^T @ attnT      (row 64 = softmax denominators)
            ctxT_norm = ctxT[0:64] * bcast(1/denom)   (GpSimd partition bcast)
  out_rev[q', e] = ctxT_norm^T-chunks @ w_o^T + b_o ; host un-reverses rows.

Performance structure:
  - fp16 operands halve HBM reads; exp(amt) multiply instead of amt add keeps
    DVE in 2x fast mode and off the PSUM port
  - every DMA costs ~2us of serial ring latency, so tensors are packed into
    few large transfers: one "early" pack (qT,kT,wqk0,emt0,biases) on the
    scalar HWDGE ring in parallel with one vT+wv pack on the sync ring, then
    one merged wqk_i+emt_i pack per chunk
  - V-projection is emitted inside chunks 0/1 (first needed by ctx0 in chunk
    2), so attention starts as soon as the early pack lands
  - per-chunk order [Q-proj, K-proj, ctx(h-4), ctx(h-3), scores(h), scores(h+1)]
    covers the K bias-add and exp latencies with PE work
  - warm-up matmuls bridge the DMA lead-in and keep the HAM clock-gate
    ramping (idle gaps re-throttle the PE to half clock for >13us)
"""
import numpy as np

import concourse.bass as bass
import concourse.tile as tile
from concourse import bacc, mybir
from concourse.bass_utils import run_bass_kernel_spmd

S = 512
D = 1024
H = 16
DK = 64
N_CORES = 8
NCH = D // 128  # 8 d-model chunks of 128
SB = S // 128   # 4 seq blocks of 128
F32 = mybir.dt.float32
F16 = mybir.dt.float16

# per-partition fp16 element offsets inside the "earlyA"/"earlyB" packs
_QT_OFF = 0
_WQK_OFF = _QT_OFF + NCH * S         # 4096
_BQK_OFF = _WQK_OFF + 2 * D          # 6144
_EARLYA_N = _BQK_OFF + 2 * NCH       # 6160
_KT_OFF = 0
_EMT_OFF = _KT_OFF + NCH * S         # 4096
_EARLYB_N = _EMT_OFF + 4 * D         # 8192

_CACHE = {}


def _build_program(with_bias=False):
    nc = bacc.Bacc("TRN2", target_bir_lowering=False, debug=False,
                   num_devices=N_CORES)

    earlyA = nc.dram_tensor("earlyA", [128, _EARLYA_N], F16,
                            kind="ExternalInput").ap()
    earlyB = nc.dram_tensor("earlyB", [128, _EARLYB_N], F16,
                            kind="ExternalInput").ap()
    vw = nc.dram_tensor("vw", [128, 3 * NCH * S], F16, kind="ExternalInput").ap()
    # wqke[i-1] for chunks 1..7: wqk_i ++ emt_i
    wqke = nc.dram_tensor("wqke", [NCH - 1, 128, 6 * D], F16,
                          kind="ExternalInput").ap()
    wo = nc.dram_tensor("wo", [128, NCH, D], F16, kind="ExternalInput").ap()
    bvo = nc.dram_tensor("bvo", [1, 2, D], F16, kind="ExternalInput").ap()
    # [p, sb, e]: row q' = sb*128 + p of the reversed output; host transposes
    out = nc.dram_tensor("out", [128, SB, D], F16, kind="ExternalOutput").ap()

    from contextlib import ExitStack

    with tile.TileContext(nc) as tc, ExitStack() as ctx:
        singles = ctx.enter_context(tc.tile_pool(name="singles", bufs=1))
        wepool = ctx.enter_context(tc.tile_pool(name="wepool", bufs=3))
        attnpool = ctx.enter_context(tc.tile_pool(name="attnpool", bufs=16))
        rcpool = ctx.enter_context(tc.tile_pool(name="rcpool", bufs=6))
        rbpool = ctx.enter_context(tc.tile_pool(name="rbpool", bufs=3))
        crawpool = ctx.enter_context(tc.tile_pool(name="crawpool", bufs=3))
        ps_proj = ctx.enter_context(tc.tile_pool(name="ps_proj", bufs=2, space="PSUM"))
        ps_sc = ctx.enter_context(tc.tile_pool(name="ps_sc", bufs=2, space="PSUM"))
        ps_ctx = ctx.enter_context(tc.tile_pool(name="ps_ctx", bufs=2, space="PSUM"))

        # Two "early" packs first on the sync HWDGE ring (it starts sooner
        # and runs faster than the scalar ring): Q-projection can begin as
        # soon as pack A lands, K/scores when B lands. The vT+wv pack rides
        # the scalar ring in parallel (V isn't consumed until ~10us after
        # attention starts).
        earlyA_sb = singles.tile([128, _EARLYA_N], F16, tag="earlyA")
        nc.sync.dma_start(out=earlyA_sb, in_=earlyA)
        earlyB_sb = singles.tile([128, _EARLYB_N], F16, tag="earlyB")
        nc.sync.dma_start(out=earlyB_sb, in_=earlyB)
        bvo_sb = singles.tile([1, 2, D], F16, tag="bvo")
        nc.scalar.dma_start(out=bvo_sb, in_=bvo)
        qT_sb = earlyA_sb[:, _QT_OFF:_WQK_OFF].rearrange("p (c s) -> p c s", s=S)
        kT_sb = earlyB_sb[:, _KT_OFF:_EMT_OFF].rearrange("p (c s) -> p c s", s=S)
        bqk_sb = earlyA_sb[:, _BQK_OFF:_EARLYA_N].rearrange(
            "p (b c) -> p b c", c=NCH)
        bq_sb, bk_sb = bqk_sb[:, 0, :], bqk_sb[:, 1, :]
        bvr_sb, bor_sb = bvo_sb[:, 0, :], bvo_sb[:, 1, :]
        wqk_sb = [None] * NCH
        emt_tiles = [None] * NCH
        wqk_sb[0] = earlyA_sb[:, _WQK_OFF:_BQK_OFF].rearrange(
            "p (w e) -> p w e", w=2)
        emt_tiles[0] = earlyB_sb[:, _EMT_OFF:_EARLYB_N].rearrange(
            "p (a b e) -> p a b e", a=2, b=2)

        ones_sb = singles.tile([1, 128], F16, tag="ones")
        nc.vector.memset(ones_sb, 1.0)
        # preload the Exp activation table while ACT is otherwise idle so the
        # first real exp doesn't pay the ~1.3us table load
        exp_warm = singles.tile([1, 16], F16, tag="expwarm")
        nc.scalar.activation(exp_warm, ones_sb[:, :16],
                             mybir.ActivationFunctionType.Exp)

        # HAM warm-up: throwaway matmuls while the input DMAs stream, so the
        # PE clock-gate ramp starts counting immediately; sized to bridge
        # until early pack A has landed.
        for _ in range(48):
            pd = ps_proj.tile([128, 512], F32, tag="proj")
            nc.tensor.matmul(pd[:, :128], lhsT=ones_sb, rhs=ones_sb,
                             start=True, stop=True)

        # ---- remaining bulk loads, in exact consumption order ----
        vw_sb = singles.tile([128, 3 * NCH * S], F16, tag="vw")
        nc.scalar.dma_start(out=vw_sb, in_=vw)
        vT_sb = vw_sb[:, 0:NCH * S].rearrange("p (c s) -> p c s", s=S)
        wv_sb = vw_sb[:, NCH * S:3 * NCH * S].rearrange(
            "p (eh c s) -> p eh c s", eh=2, s=S)
        for i in range(1, NCH):
            t = wepool.tile([128, 6 * D], F16, tag="we")
            nc.sync.dma_start(out=t, in_=wqke[i - 1])
            wqk_sb[i] = t[:, 0:2 * D].rearrange("p (w e) -> p w e", w=2)
            emt_tiles[i] = t[:, 2 * D:6 * D].rearrange(
                "p (a b e) -> p a b e", a=2, b=2)
        wo_sb = singles.tile([128, NCH, D], F16, tag="wo")
        nc.sync.dma_start(out=wo_sb, in_=wo)

        # big persistent activations
        QT_sb = singles.tile([128, NCH, S], F16, tag="QT")
        KT_sb = singles.tile([128, NCH, S], F16, tag="KT")
        # V with a ones column appended per head: [128, sb, 16*65]
        V_sb = singles.tile([128, SB, H * (DK + 1)], F16, tag="V")
        ctxT_sb = singles.tile([128, NCH, S], F16, tag="ctxT")
        ones_col = singles.tile([128, H], F16, tag="ones_col")
        nc.vector.memset(ones_col, 1.0)

        # ---- V projection, emitted inside chunks 0/1: V[s,e] = vT^T@wv + b_v
        for sb in range(SB):
            v_heads = V_sb[:, sb, :].rearrange("p (h c) -> p h c", c=DK + 1)
            nc.scalar.copy(v_heads[:, :, DK], ones_col)

        def emit_vproj(eh):
            for sb in range(SB):
                pv = ps_proj.tile([128, 512], F32, tag="proj")
                for dc in range(NCH):
                    nc.tensor.matmul(
                        pv,
                        lhsT=vT_sb[:, dc, sb * 128:(sb + 1) * 128],
                        rhs=wv_sb[:, eh, dc, :],
                        start=(dc == 0), stop=(dc == NCH - 1 and not with_bias),
                    )
                if with_bias:
                    nc.tensor.matmul(
                        pv, lhsT=ones_sb,
                        rhs=bvr_sb[:, eh * 512:(eh + 1) * 512],
                        start=False, stop=True,
                    )
                v_heads = V_sb[:, sb, :].rearrange("p (h c) -> p h c", c=DK + 1)
                # alternate the PSUM evacuation between ACT and DVE so the
                # copies don't queue behind the early heads' exps
                dst = v_heads[:, 8 * eh:8 * eh + 8, 0:DK]
                src = pv.rearrange("p (h d) -> p h d", d=DK)
                if sb % 2 == 0:
                    nc.scalar.copy(dst, src)
                else:
                    nc.vector.tensor_scalar_add(dst, src, 0.0)

        # ---- interleaved Q/K projection chunks + attention heads ----
        def emit_scores(h):
            i, p0 = h // 2, (h % 2) * 64
            emt_h = emt_tiles[i][:, h % 2, :, :]
            QT_h = QT_sb[p0:p0 + 64, i, :]
            attn_tiles = []
            for pair in range(2):
                ps = ps_sc.tile([128, 2 * S], F32, tag="pair")
                for j in range(2):
                    kb = 2 * pair + j
                    nc.tensor.matmul(
                        ps[:, j * 512:(j + 1) * 512],
                        lhsT=KT_sb[p0:p0 + 64, i, kb * 128:(kb + 1) * 128],
                        rhs=QT_h, start=True, stop=True,
                    )
                ate = attnpool.tile([128, 2 * S], F16, tag="attn")
                nc.scalar.activation(ate, ps, mybir.ActivationFunctionType.Exp)
                at = attnpool.tile([128, 2 * S], F16, tag="attn")
                nc.vector.tensor_mul(at, ate, emt_h[:, pair, :])
                attn_tiles.append(at)
            return attn_tiles

        def emit_ctx(h, attn_tiles):
            i, p0 = h // 2, (h % 2) * 64
            pc = ps_ctx.tile([DK + 1, 512], F32, tag="ctx")
            for kb in range(SB):
                nc.tensor.matmul(
                    pc, lhsT=V_sb[:, kb, h * 65:(h + 1) * 65],
                    rhs=attn_tiles[kb // 2][:, (kb % 2) * 512:(kb % 2 + 1) * 512],
                    start=(kb == 0), stop=(kb == SB - 1),
                )
            # Evacuate PSUM immediately (sums on DVE, raw context on ACT) so
            # the bank recycles in ~1.5us; the reciprocal/broadcast/normalize
            # chain then runs entirely off-PSUM and off the critical path.
            sums_sb = rcpool.tile([1, 512], F32, tag="recip")
            nc.vector.tensor_scalar_add(sums_sb, pc[DK:DK + 1, :], 0.0)
            craw = crawpool.tile([64, 512], F32, tag="craw")
            nc.scalar.copy(craw, pc[0:DK, :])
            recip_f32 = rcpool.tile([1, 512], F32, tag="recip")
            nc.vector.reciprocal_approx_fast(out=recip_f32, in_=sums_sb)
            # broadcast 1/denom and normalize on GpSimd (its queue does
            # nothing else, and all three operands live in SBUF) — keeps both
            # ops off the saturated ACT/DVE queues
            r_sb = rbpool.tile([64, 512], F32, tag="rbc")
            nc.gpsimd.partition_broadcast(r_sb, recip_f32, channels=64)
            nc.vector.tensor_mul(ctxT_sb[p0:p0 + 64, i, :], craw, r_sb)

        # Per-chunk emission: chunks 0/1 carry the V-projection halves (V is
        # first needed by ctx0 in chunk 2); later chunks carry two context
        # groups, which also cover the K bias-add drain. Each head's exp+mask
        # gets more than a chunk of PE work before its context matmuls run.
        pending = []  # [(head, attn_tiles)] awaiting context matmuls
        for i in range(NCH):  # e-chunk i covers heads 2i, 2i+1
            pq = ps_proj.tile([128, 512], F32, tag="proj")
            for dc in range(NCH):
                nc.tensor.matmul(
                    pq, lhsT=wqk_sb[i][:, 0, dc * 128:(dc + 1) * 128],
                    rhs=qT_sb[:, dc, :],
                    start=(dc == 0), stop=(dc == NCH - 1),
                )
            nc.scalar.add(QT_sb[:, i, :], pq, bq_sb[:, i:i + 1])
            if i == 0:
                # bridge the earlyA->earlyB DMA window with throwaway
                # matmuls so the HAM activity window keeps counting
                for _ in range(10):
                    pd = ps_proj.tile([128, 512], F32, tag="proj")
                    nc.tensor.matmul(pd[:, :128], lhsT=ones_sb, rhs=ones_sb,
                                     start=True, stop=True)
            pk = ps_proj.tile([128, 512], F32, tag="proj")
            for dc in range(NCH):
                nc.tensor.matmul(
                    pk, lhsT=wqk_sb[i][:, 1, dc * 128:(dc + 1) * 128],
                    rhs=kT_sb[:, dc, :],
                    start=(dc == 0), stop=(dc == NCH - 1),
                )
            nc.scalar.add(KT_sb[:, i, :], pk, bk_sb[:, i:i + 1])

            if i >= 2:
                n = 3 if i >= 6 else 2
                for p in pending[:n]:
                    emit_ctx(*p)
                pending = pending[n:]
            for sub in range(2):
                h = 2 * i + sub
                pending.append((h, emit_scores(h)))
            if i < 2:
                emit_vproj(i)
        for p in pending:
            emit_ctx(*p)

        # ---- output projection: out_rev[q', e] = ctxT^T @ wo + b_o ----
        # sb0 runs its ch7 (heads 14/15) contractions last so the final
        # head's normalization has PE work as cover instead of stalling
        osb_all = singles.tile([128, SB, D], F16, tag="osb")
        for sb in range(SB):
            po = ps_sc.tile([128, 2 * S], F32, tag="pair")
            if sb == 0:
                for eh in range(2):
                    for ch in range(NCH - 1):
                        nc.tensor.matmul(
                            po[:, eh * 512:(eh + 1) * 512],
                            lhsT=ctxT_sb[:, ch, 0:128],
                            rhs=wo_sb[:, ch, eh * 512:(eh + 1) * 512],
                            start=(ch == 0), stop=False,
                        )
                for eh in range(2):
                    nc.tensor.matmul(
                        po[:, eh * 512:(eh + 1) * 512],
                        lhsT=ctxT_sb[:, NCH - 1, 0:128],
                        rhs=wo_sb[:, NCH - 1, eh * 512:(eh + 1) * 512],
                        start=False, stop=not with_bias,
                    )
                    if with_bias:
                        nc.tensor.matmul(
                            po[:, eh * 512:(eh + 1) * 512], lhsT=ones_sb,
                            rhs=bor_sb[:, eh * 512:(eh + 1) * 512],
                            start=False, stop=True,
                        )
            else:
                for eh in range(2):
                    half = po[:, eh * 512:(eh + 1) * 512]
                    for ch in range(NCH):
                        nc.tensor.matmul(
                            half, lhsT=ctxT_sb[:, ch, sb * 128:(sb + 1) * 128],
                            rhs=wo_sb[:, ch, eh * 512:(eh + 1) * 512],
                            start=(ch == 0),
                            stop=(ch == NCH - 1 and not with_bias),
                        )
                    if with_bias:
                        nc.tensor.matmul(
                            half, lhsT=ones_sb,
                            rhs=bor_sb[:, eh * 512:(eh + 1) * 512],
                            start=False, stop=True,
                        )
            # alternate the PSUM evacuation between ACT and DVE so the four
            # copies don't serialize on one engine at the tail
            if sb % 2 == 0:
                nc.scalar.copy(osb_all[:, sb, :], po)
            else:
                nc.vector.tensor_scalar_add(osb_all[:, sb, :], po, 0.0)
            # ship each row-block as soon as its copy lands so the transfers
            # overlap the remaining compute
            nc.sync.dma_start(out=out[:, sb, :], in_=osb_all[:, sb, :])

    nc.compile()
    return nc


def _prep_inputs(query, key, value, mask, w_q, b_q, w_k, b_k, w_v, b_v,
                 w_o, b_o, rel_bias):
    query = np.asarray(query, np.float32)
    key = np.asarray(key, np.float32)
    value = np.asarray(value, np.float32)
    mask = np.asarray(mask)
    w_q = np.asarray(w_q, np.float32)
    w_k = np.asarray(w_k, np.float32)
    w_v = np.asarray(w_v, np.float32)
    w_o = np.asarray(w_o, np.float32)
    b_q = np.asarray(b_q, np.float32)
    b_k = np.asarray(b_k, np.float32)
    b_v = np.asarray(b_v, np.float32)
    b_o = np.asarray(b_o, np.float32)
    rel_bias = np.asarray(rel_bias, np.float32)

    def chunked_T(w):
        # wc[i, p, dc*128+e] = w.T[dc*128+p, i*128+e]
        wt = np.ascontiguousarray(w.T).reshape(NCH, 128, NCH, 128)
        return wt.transpose(2, 1, 0, 3).reshape(NCH, 128, D)

    def part_major(xT_cols):
        # [1024, 512] -> [128, 8*512] with row dc*128+p at [p, dc]
        return xT_cols.reshape(NCH, 128, S).transpose(1, 0, 2).reshape(128, NCH * S)

    wqk = np.stack([chunked_T(w_q / 8.0), chunked_T(w_k)], axis=2)  # [i,p,2,D]
    wqk = wqk.reshape(NCH, 128, 2 * D).astype(np.float16)
    wv_h = w_v.T.reshape(NCH, 128, 2, S).transpose(1, 2, 0, 3).reshape(128, 2 * NCH * S)
    bqk16 = np.stack([(b_q / 8.0).reshape(NCH, 128).T,
                      b_k.reshape(NCH, 128).T], axis=1)  # [128, 2, NCH]

    # biasT_rev[h, k, q'] = rel_bias[k + q', h]; emt = exp(bias) * mask01
    idx = np.arange(S)[:, None] + np.arange(S)[None, :]  # [k, q'] in [0, 1022]
    ebias_t = np.exp(rel_bias)[idx]        # [S, S, H]
    ebias_t = np.ascontiguousarray(ebias_t.transpose(2, 0, 1))  # [H, k, q']

    shared = {
        "wqke": np.ascontiguousarray(
            np.concatenate([wqk[1:], np.zeros((NCH - 1, 128, 4 * D), np.float16)],
                           axis=2)),
        "wo": np.ascontiguousarray(
            w_o.T.reshape(NCH, 128, D).transpose(1, 0, 2).astype(np.float16)),
        "bvo": np.stack([b_v, b_o]).reshape(1, 2, D).astype(np.float16),
    }

    in_maps = []
    for c in range(N_CORES):
        # maskT_rev[k, q'] multiplicative: mask[c, 0, 511-q', k] in {0, 1}
        m01 = mask[c, 0][::-1, :].T.astype(np.float32)   # [k, q']
        a = (ebias_t * m01[None]).astype(np.float16)     # [H, k, q']
        # [h=2pr+sub, k=(2pair+j)*128+p, q] -> [pr, p, sub*pair*j*q = 4*D]
        a = a.reshape(NCH, 2, 2, 2, 128, S).transpose(0, 4, 1, 2, 3, 5)
        a = np.ascontiguousarray(a).reshape(NCH, 128, 4 * D)

        earlyA_c = np.empty((128, _EARLYA_N), np.float16)
        earlyA_c[:, _QT_OFF:_WQK_OFF] = part_major(
            query[c].T[:, ::-1].astype(np.float16))
        earlyA_c[:, _WQK_OFF:_BQK_OFF] = wqk[0]
        earlyA_c[:, _BQK_OFF:_EARLYA_N] = bqk16.reshape(128, 2 * NCH)
        earlyB_c = np.empty((128, _EARLYB_N), np.float16)
        earlyB_c[:, _KT_OFF:_EMT_OFF] = part_major(key[c].T.astype(np.float16))
        earlyB_c[:, _EMT_OFF:_EARLYB_N] = a[0]

        vw_c = np.empty((128, 3 * NCH * S), np.float16)
        vw_c[:, 0:NCH * S] = part_major(value[c].T.astype(np.float16))
        vw_c[:, NCH * S:] = wv_h

        im = dict(shared)
        im["wqke"] = im["wqke"].copy()
        im["wqke"][:, :, 2 * D:] = a[1:]
        im["earlyA"] = earlyA_c
        im["earlyB"] = earlyB_c
        im["vw"] = vw_c
        in_maps.append(im)
    return in_maps


def kernel(query, key, value, mask, w_q, b_q, w_k, b_k, w_v, b_v, w_o, b_o,
           rel_bias, _run_opts=None):
    # b_v/b_o fold into extra K=1 matmuls only when actually nonzero
    with_bias = bool(np.any(np.asarray(b_v)) or np.any(np.asarray(b_o)))
    key_ = ("nc", with_bias)
    if key_ not in _CACHE:
        _CACHE[key_] = _build_program(with_bias)
    nc = _CACHE[key_]
    in_maps = _prep_inputs(query, key, value, mask, w_q, b_q, w_k, b_k,
                           w_v, b_v, w_o, b_o, rel_bias)
    opts = _run_opts or {}
    res = run_bass_kernel_spmd(nc, in_maps, list(range(N_CORES)), **opts)
    out = np.stack([
        res.results[c]["out"].transpose(1, 0, 2).reshape(S, D)[::-1, :]
        for c in range(N_CORES)
    ])
    if _run_opts is not None:
        _CACHE["last_result"] = res
    return out.astype(np.float32)
